# revision 7
# baseline (speedup 1.0000x reference)
"""Fused TP all-reduce + residual add + RMSNorm for Trainium2.

Problem: hidden_states [4, 4096, 7168] f32 (per-rank row-parallel GEMM
partials), residual [4096, 7168] f32, norm_weight [7168] f32.
  reduced      = sum(hidden_states, axis=0)
  residual_out = reduced + residual
  normed       = residual_out * rsqrt(mean(residual_out^2, -1) + eps) * norm_weight
Returns (normed, residual_out).

Strategy: since kernel() receives the FULL inputs, shard over tokens
(4096 / 8 cores = 512 tokens per core) and hand each core all 4 partials
for its token slab. The "all-reduce" degenerates to 4 local elementwise
adds per core — no cross-core collective at all — and the kernel is
purely HBM-bandwidth-bound (~103 MB of DMA per core).
"""

import numpy as np

import concourse.bacc as bacc
import concourse.bass as bass
import concourse.tile as tile
from concourse import mybir
from concourse.bass_utils import run_bass_kernel_spmd

TP = 4
TOKENS = 4096
HIDDEN = 7168
EPS = 1e-6
N_CORES = 8
TOK = TOKENS // N_CORES  # 512 tokens per core
P = 128                  # SBUF partitions
NT = TOK // P            # 4 row-tiles per core
F32 = mybir.dt.float32

_NC_CACHE = {}


def _build_nc() -> bass.Bass:
    nc = bacc.Bacc("TRN2", target_bir_lowering=False, debug=False)
    h = nc.dram_tensor("h", [TP, TOK, HIDDEN], F32, kind="ExternalInput")
    res = nc.dram_tensor("res", [TOK, HIDDEN], F32, kind="ExternalInput")
    w = nc.dram_tensor("w", [HIDDEN], F32, kind="ExternalInput")
    normed = nc.dram_tensor("normed", [TOK, HIDDEN], F32, kind="ExternalOutput")
    res_out = nc.dram_tensor("res_out", [TOK, HIDDEN], F32, kind="ExternalOutput")

    with tile.TileContext(nc) as tc:
        with (
            tc.tile_pool(name="singles", bufs=1) as singles,
            tc.tile_pool(name="loads", bufs=3) as loads,
            tc.tile_pool(name="rows", bufs=2) as rows,
            tc.tile_pool(name="stats", bufs=4) as stats,
        ):
            # norm_weight replicated across all 128 partitions, loaded once
            w_tile = singles.tile([P, HIDDEN], F32)
            w_ap = w[:]
            w_bcast = bass.AP(
                tensor=w_ap.tensor, offset=w_ap.offset, ap=[[0, P], w_ap.ap[0]]
            )
            nc.gpsimd.dma_start(out=w_tile, in_=w_bcast)
            eps_t = singles.tile([P, 1], F32)
            nc.vector.memset(eps_t, EPS)

            for t in range(NT):
                sl = slice(t * P, (t + 1) * P)
                # r accumulates residual_out = residual + sum_p h[p]
                r = rows.tile([P, HIDDEN], F32, tag="r")
                nc.sync.dma_start(out=r, in_=res[sl, :])
                hp = [
                    loads.tile([P, HIDDEN], F32, tag="h", name=f"h{p}")
                    for p in range(TP)
                ]
                for p in range(TP):
                    nc.sync.dma_start(out=hp[p], in_=h[p, sl, :])
                for p in range(TP):
                    nc.vector.tensor_add(out=r, in0=r, in1=hp[p])

                # residual_out is final — store it
                nc.sync.dma_start(out=res_out[sl, :], in_=r)

                # mean/var over the 7168 free dim via bn_stats subgroups of 512
                n_sub = HIDDEN // 512  # 14
                r_g = r.rearrange("p (s f) -> p s f", f=512)
                st = stats.tile([P, n_sub, 6], F32, tag="st")
                for s in range(n_sub):
                    nc.vector.bn_stats(out=st[:, s, :], in_=r_g[:, s, :])
                mv = stats.tile([P, 2], F32, tag="mv")
                nc.vector.bn_aggr(out=mv, in_=st)
                # mean(r^2) = var + mean^2
                meansq = stats.tile([P, 1], F32, tag="meansq")
                nc.vector.tensor_mul(out=meansq, in0=mv[:, 0:1], in1=mv[:, 0:1])
                nc.vector.tensor_add(out=meansq, in0=meansq, in1=mv[:, 1:2])
                # rstd = 1 / sqrt(meansq + eps)
                rstd = stats.tile([P, 1], F32, tag="rstd")
                nc.scalar.activation(
                    out=rstd,
                    in_=meansq,
                    func=mybir.ActivationFunctionType.Sqrt,
                    bias=eps_t,
                    scale=1.0,
                )
                nc.vector.reciprocal(out=rstd, in_=rstd)

                # normed = r * rstd * w (both on DVE)
                n = loads.tile([P, HIDDEN], F32, tag="h")
                nc.vector.tensor_scalar_mul(out=n, in0=r, scalar1=rstd)
                nc.vector.tensor_mul(out=n, in0=n, in1=w_tile)
                nc.sync.dma_start(out=normed[sl, :], in_=n)

    nc.compile()
    return nc


def _get_nc() -> bass.Bass:
    if "nc" not in _NC_CACHE:
        _NC_CACHE["nc"] = _build_nc()
    return _NC_CACHE["nc"]


def _make_in_maps(hidden_states, residual, norm_weight):
    hidden_states = np.ascontiguousarray(hidden_states, dtype=np.float32)
    residual = np.ascontiguousarray(residual, dtype=np.float32)
    norm_weight = np.ascontiguousarray(norm_weight, dtype=np.float32)
    in_maps = []
    for c in range(N_CORES):
        sl = slice(c * TOK, (c + 1) * TOK)
        in_maps.append(
            {
                "h": np.ascontiguousarray(hidden_states[:, sl, :]),
                "res": np.ascontiguousarray(residual[sl, :]),
                "w": norm_weight,
            }
        )
    return in_maps


def _run(in_maps, **kwargs):
    return run_bass_kernel_spmd(
        _get_nc(), in_maps, core_ids=list(range(N_CORES)), **kwargs
    )


def _assemble(results):
    normed = np.concatenate([r["normed"] for r in results], axis=0)
    res_out = np.concatenate([r["res_out"] for r in results], axis=0)
    return normed, res_out


def kernel(hidden_states, residual, norm_weight):
    in_maps = _make_in_maps(hidden_states, residual, norm_weight)
    out = _run(in_maps)
    return _assemble(out.results)
